# revision 33
# baseline (speedup 1.0000x reference)
"""DevignLite GNN (3-layer GCN + dual pooling + MLP head) on 8 Trainium2 NeuronCores.

Strategy (v2)
-------------
- Nodes (and incident edges, bucketed by dst) are partitioned across 8 cores.
  Per GCN layer, with separable GCN normalization norm(e) = dinv[src]*dinv[dst]:
      u = dinv * x                     (node-local scale)
      z[v] = u[v] + sum_{e: dst=v} u[src_e]   (self-loop + edge aggregation)
      x' = relu((dinv * z) @ W + b)
- u is replicated across cores via quarter AllGathers (int16-indexable shard
  views); per-edge u[src] rows are fetched with SWDGE dma_gather calls spread
  round-robin over 4 SWDGE queues, with calls interleaved across the 4 shard
  quarters so the d-major consumer can run close behind all queues.
- Aggregation: per 128-edge tile a one-hot S (DVE is_equal of iota vs local
  dst ids) feeds psum[dst,64] += S.T @ msg on the PE.  All shards' tiles of a
  dst block accumulate into ONE psum tile, initialized with the self-loop
  contribution via an identity matmul of the SBUF-resident u table (self-loop
  edges are excluded from the gather stream entirely).
- Per-node update per dst block: dinv scale, transpose, W matmul, biased relu,
  transpose back; the next-layer u rows go to a persistent SBUF table (for the
  next layer's self-loop init) and to DRAM for the AllGather.
- Pooling: segment mean via one-hot matmul with a ones column; segment max via
  segmented max-scan + one-hot extraction of segment-end columns.  Partials
  exchanged with a small AllGather; every core computes the classifier head.
"""

import os
import sys

sys.path.insert(0, "/opt/trn_rl_repo")

import numpy as np

P = 128
D = 64


class Cfg:
    def __init__(self, N, E, V, G, n_cores=8, call_tiles=16):
        self.N, self.E, self.V, self.G = N, E, V, G
        self.NC = n_cores
        assert N % n_cores == 0
        self.NL = N // n_cores                      # nodes per core
        self.NLP = -(-self.NL // P) * P             # padded to 128
        self.DB = self.NLP // P                     # dst blocks per core
        # quarter-major shard views: split each rank's rows into NQ quarters
        # (tile-aligned); shard q's table = all ranks' quarter-q rows.
        NQ = 4
        base = self.DB // NQ
        rem = self.DB % NQ
        qtiles = [base + (1 if i < rem else 0) for i in range(NQ)]
        self.qtiles = [q for q in qtiles if q > 0]
        self.NSH = len(self.qtiles)
        self.qrows = [q * P for q in self.qtiles]
        self.qstart = [0]
        for q in self.qrows[:-1]:
            self.qstart.append(self.qstart[-1] + q)
        for q in self.qrows:
            assert n_cores * q < 32768, "quarter shard exceeds int16 range"
        self.VBS = min(V, 25000)                    # vocab block size
        self.NVB = -(-V // self.VBS)
        self.GB = -(-G // P)                        # graph blocks (pool)
        self.CALL_TILES = call_tiles


# ----------------------------------------------------------------------------
# host-side preprocessing (structure only: bucketing, index streams, degrees)
# ----------------------------------------------------------------------------
def _preprocess(cfg, x_tokens, edge_index, batch):
    c = cfg
    N, NC, NL, NLP, DB, NSH = c.N, c.NC, c.NL, c.NLP, c.DB, c.NSH

    src = np.asarray(edge_index[0], dtype=np.int64)
    dst = np.asarray(edge_index[1], dtype=np.int64)
    # degree includes self loops (GCN adds them); self-loop messages are
    # folded in on-chip from the local u table, not gathered.
    deg = (np.bincount(dst, minlength=N) + 1.0).astype(np.float32)

    src_rank = src // NL
    src_loc = src % NL
    qstart_arr = np.asarray(c.qstart, dtype=np.int64)
    qrows_arr = np.asarray(c.qrows, dtype=np.int64)
    shard = np.searchsorted(qstart_arr, src_loc, side="right") - 1
    idx16 = (src_rank * qrows_arr[shard] + (src_loc - qstart_arr[shard])).astype(
        np.int64
    )
    edge_core = dst // NL
    ldst = dst % NL
    db = ldst // P
    lid = ldst % P
    cell = shard * DB + db

    NCELL = NSH * DB
    counts = np.zeros((NC, NCELL), dtype=np.int64)
    percore = []
    sort_src = bool(int(os.environ.get("K_SORT", "0")))
    for ci in range(NC):
        m = edge_core == ci
        cc = cell[m]
        if sort_src:
            order = np.lexsort((idx16[m], cc))
        else:
            order = np.argsort(cc, kind="stable")
        counts[ci] = np.bincount(cc, minlength=NCELL)
        percore.append((cc[order], idx16[m][order], lid[m][order]))

    slot = -(-counts.max(axis=0) // P)               # tiles per cell, shared
    tiles_per_shard = slot.reshape(NSH, DB).sum(axis=1)
    cell_tile_start = np.zeros(NCELL, dtype=np.int64)  # within-shard tile idx
    for s in range(NSH):
        cum = 0
        for d in range(DB):
            cell_tile_start[s * DB + d] = cum
            cum += slot[s * DB + d]
    NT_TOT = int(slot.sum())

    # calls: per shard, chunks of CALL_TILES tiles; interleaved round-robin
    # across shards so d-major consumption tracks all 4 queues.
    percall = []
    for s in range(NSH):
        rem = int(tiles_per_shard[s])
        off = 0
        chunks = []
        while rem > 0:
            nt = min(c.CALL_TILES, rem)
            chunks.append((s, off, nt))
            off += nt
            rem -= nt
        percall.append(chunks)
    calls = []
    while any(percall):
        for s in range(NSH):
            if percall[s]:
                calls.append(percall[s].pop(0))
    # map (shard, within-shard tile) -> (call index, col within call)
    tile2call = {}
    for cidx, (s, off, nt) in enumerate(calls):
        for j in range(nt):
            tile2call[(s, off + j)] = (cidx, j)

    # cells schedule, d-major: for each d, the NSH shard cells
    cells_sched = []                                 # (d, [(s, t0, nt), ...])
    for d in range(DB):
        ents = []
        for s in range(NSH):
            sl = int(slot[s * DB + d])
            if sl:
                ents.append((s, int(cell_tile_start[s * DB + d]), sl))
        cells_sched.append((d, ents))

    # per-core edge index / dst-id streams, laid out per shard stream
    shard_t0 = np.concatenate([[0], np.cumsum(tiles_per_shard)[:-1]])
    edge_idx_all = np.zeros((NC, NT_TOT * P), dtype=np.int16)
    edge_ids_all = np.full((NC, NT_TOT * P), -1.0, dtype=np.float32)
    for ci in range(NC):
        cc, ii, ll = percore[ci]
        within = np.arange(cc.size) - np.concatenate(
            [[0], np.cumsum(counts[ci])[:-1]]
        )[cc]
        s_of = cc // DB
        pos = (shard_t0[s_of] + cell_tile_start[cc]) * P + within
        edge_idx_all[ci, pos] = ii.astype(np.int16)
        edge_ids_all[ci, pos] = ll.astype(np.float32)

    def wrap_idx(a):                                 # [n] -> [128, n/16] int16
        n = a.size
        assert n % 16 == 0
        w = a.reshape(n // 16, 16).T
        return np.tile(w, (8, 1)).astype(np.int16)

    def tile_layout(a, fill, ncols):                 # [n] -> [128, ncols]
        out = np.full((P, ncols), fill, dtype=np.float32)
        n = a.size
        t = np.arange(n) // P
        p = np.arange(n) % P
        out[p, t] = a
        return out

    edge_idx_w = np.stack([wrap_idx(edge_idx_all[ci]) for ci in range(NC)])
    edge_ids_t = np.stack(
        [tile_layout(edge_ids_all[ci], -1.0, NT_TOT) for ci in range(NC)]
    )

    # --- embedding gather / scatter streams -------------------------------
    toks = np.asarray(x_tokens, dtype=np.int64).reshape(-1)
    vb = toks // c.VBS
    emb_cnt = np.zeros((NC, c.NVB), dtype=np.int64)
    for ci in range(NC):
        emb_cnt[ci] = np.bincount(vb[ci * NL : (ci + 1) * NL], minlength=c.NVB)
    EC = int(-(-emb_cnt.max() // P) * P)
    TRASH = NLP
    tok_idx = np.zeros((NC, c.NVB * EC), dtype=np.int16)
    tok_scat = np.full((NC, c.NVB * EC), TRASH, dtype=np.int16)
    deg_perm = np.ones((NC, c.NVB * EC), dtype=np.float32)
    for ci in range(NC):
        tl = toks[ci * NL : (ci + 1) * NL]
        dl = deg[ci * NL : (ci + 1) * NL]
        vbl = vb[ci * NL : (ci + 1) * NL]
        for b in range(c.NVB):
            rows = np.nonzero(vbl == b)[0]
            o = b * EC
            tok_idx[ci, o : o + rows.size] = (tl[rows] % c.VBS).astype(np.int16)
            tok_scat[ci, o : o + rows.size] = rows.astype(np.int16)
            deg_perm[ci, o : o + rows.size] = dl[rows]

    tok_idx_w = np.stack([wrap_idx(tok_idx[ci]) for ci in range(NC)])
    tok_scat_w = np.stack([wrap_idx(tok_scat[ci]) for ci in range(NC)])
    ECC = EC // P
    deg_perm_t = np.stack(
        [
            np.concatenate(
                [
                    tile_layout(deg_perm[ci, b * EC : (b + 1) * EC], 1.0, ECC)
                    for b in range(c.NVB)
                ],
                axis=1,
            )
            for ci in range(NC)
        ]
    )

    # --- per-node degree / graph metadata ---------------------------------
    batch = np.asarray(batch, dtype=np.int64)
    deg_loc = np.ones((NC, P, DB), dtype=np.float32)
    g_ids = np.full((NC, P, DB), -1.0, dtype=np.float32)
    is_end = np.zeros((NC, P, DB), dtype=np.float32)
    for ci in range(NC):
        dl = deg[ci * NL : (ci + 1) * NL]
        bl = batch[ci * NL : (ci + 1) * NL].astype(np.float32)
        e = np.zeros(NL, dtype=np.float32)
        if NL > 1:
            e[:-1] = (bl[1:] != bl[:-1]).astype(np.float32)
        e[-1] = 1.0
        deg_loc[ci] = tile_layout(dl, 1.0, DB)
        g_ids[ci] = tile_layout(bl, -1.0, DB)
        is_end[ci] = tile_layout(e, 0.0, DB)

    meta = dict(calls=calls, cells=cells_sched, tile2call=tile2call,
                shard_t0=[int(x) for x in shard_t0], NT_TOT=NT_TOT, EC=EC)
    data = dict(
        edge_idx=edge_idx_w,
        edge_ids=edge_ids_t,
        tok_idx=tok_idx_w,
        tok_scat=tok_scat_w,
        deg_perm=deg_perm_t,
        deg_loc=deg_loc,
        g_ids=g_ids,
        is_end=is_end,
    )
    return meta, data


# ----------------------------------------------------------------------------
# the Bass/Tile program
# ----------------------------------------------------------------------------
def build_program(cfg, meta):
    import concourse.bacc as bacc
    import concourse.tile as tile
    from concourse import mybir
    from concourse.masks import make_identity

    no_coll = bool(int(os.environ.get("K_NO_COLL", "0")))
    no_agg = bool(int(os.environ.get("K_NO_AGG", "0")))
    sgen_gp = int(os.environ.get("K_SGEN_GP", "0"))
    cast_dma = bool(int(os.environ.get("K_CAST_DMA", "0")))
    dve_copy = bool(int(os.environ.get("K_DVE_COPY", "0")))
    s_bufs = int(os.environ.get("K_S_BUFS", "8"))
    lin_gather = bool(int(os.environ.get("K_LIN_GATHER", "0")))
    nswq = int(os.environ.get("K_NSWQ", "4"))
    msg_bufs = int(os.environ.get("K_MSG_BUFS", "16"))
    msgf_bufs = int(os.environ.get("K_MSGF_BUFS", "8"))

    c = cfg
    f32 = mybir.dt.float32
    bf16 = mybir.dt.bfloat16
    i16 = mybir.dt.int16
    AF = mybir.ActivationFunctionType
    OP = mybir.AluOpType
    NT_TOT, EC = meta["NT_TOT"], meta["EC"]
    CALLS, CELLS, T2C = meta["calls"], meta["cells"], meta["tile2call"]
    SHARD_T0 = meta["shard_t0"]
    ECC = EC // P
    DBL = c.DB
    rg = [list(range(c.NC))]
    PCOLS = 2 * (2 * D + 1)                          # per-graph-block pool cols

    nc = bacc.Bacc("TRN2", target_bir_lowering=False, debug=False,
                   enable_asserts=False, num_devices=c.NC,
                   num_swdge_queues=nswq)

    emb = nc.dram_tensor("emb_table", [c.V, D], f32, kind="ExternalInput")
    edge_idx_d = nc.dram_tensor("edge_idx", [P, NT_TOT * 8], i16, kind="ExternalInput")
    edge_ids_d = nc.dram_tensor("edge_ids", [P, NT_TOT], f32, kind="ExternalInput")
    tok_idx_d = nc.dram_tensor("tok_idx", [P, c.NVB * EC // 16], i16, kind="ExternalInput")
    tok_scat_d = nc.dram_tensor("tok_scat", [P, c.NVB * EC // 16], i16, kind="ExternalInput")
    deg_perm_d = nc.dram_tensor("deg_perm", [P, c.NVB * ECC], f32, kind="ExternalInput")
    deg_loc_d = nc.dram_tensor("deg_loc", [P, DBL], f32, kind="ExternalInput")
    g_ids_d = nc.dram_tensor("g_ids", [P, DBL], f32, kind="ExternalInput")
    is_end_d = nc.dram_tensor("is_end", [P, DBL], f32, kind="ExternalInput")
    Ws = [nc.dram_tensor(f"W{i}", [D, D], f32, kind="ExternalInput") for i in range(3)]
    bs = [nc.dram_tensor(f"b{i}", [D], f32, kind="ExternalInput") for i in range(3)]
    Wc1_d = nc.dram_tensor("Wc1", [2 * D, D], f32, kind="ExternalInput")
    bc1_d = nc.dram_tensor("bc1", [D], f32, kind="ExternalInput")
    Wc2_d = nc.dram_tensor("Wc2", [D, 2], f32, kind="ExternalInput")
    bc2_d = nc.dram_tensor("bc2", [2], f32, kind="ExternalInput")
    logits_d = nc.dram_tensor("logits", [c.G, 2], f32, kind="ExternalOutput")

    u_loc = [
        nc.dram_tensor(f"u{i}_loc", [c.NLP + P, D], f32, kind="Internal")
        for i in range(3)
    ]
    u_full = [
        [
            nc.dram_tensor(f"u{i}_full_q{q}", [c.NC * c.qrows[q], D], f32,
                           kind="Internal", addr_space="Shared")
            for q in range(c.NSH)
        ]
        for i in range(3)
    ]
    pool_loc_d = nc.dram_tensor("pool_loc", [P, c.GB * PCOLS], f32, kind="Internal")
    pool_all_d = nc.dram_tensor("pool_all", [c.NC * P, c.GB * PCOLS], f32,
                                kind="Internal", addr_space="Shared")

    iota_f = nc.inline_tensor(
        np.tile(np.arange(P, dtype=np.float32), (P, 1)), name="iota_f"
    )

    def rsqrt_refined(dst, deg_src, tmp_pool, ncols):
        """dst = deg_src**-0.5 with one Newton step (f32-accurate)."""
        nc.scalar.sqrt(dst[:], deg_src[:])
        nc.vector.reciprocal(dst[:], dst[:])
        t = tmp_pool.tile([P, ncols], f32, tag="nwt", name=f"nwt{id(dst) % 9999}")
        nc.vector.tensor_tensor(t[:], dst[:], dst[:], OP.mult)
        nc.vector.tensor_tensor(t[:], t[:], deg_src[:], OP.mult)
        nc.vector.tensor_scalar(t[:], t[:], -0.5, 1.5, OP.mult, OP.add)
        nc.vector.tensor_tensor(dst[:], dst[:], t[:], OP.mult)

    with tile.TileContext(nc) as tc:
        with (
            tc.tile_pool(name="persist", bufs=1) as pp,
            tc.tile_pool(name="msg", bufs=msg_bufs) as msgp,
            tc.tile_pool(name="msgf", bufs=msgf_bufs) as msgfp,
            tc.tile_pool(name="emsg", bufs=2) as emsgp,
            tc.tile_pool(name="sel", bufs=s_bufs) as sp,
            tc.tile_pool(name="work", bufs=2) as wp,
            tc.tile_pool(name="ps", bufs=2, space="PSUM") as psp,
            tc.tile_pool(name="zps", bufs=4, space="PSUM") as zpsp,
            tc.tile_pool(name="ppool", bufs=1, space="PSUM") as ppsum,
        ):
            # ---------- persistent SBUF state --------------------------------
            idx_sb = pp.tile([P, NT_TOT * 8], i16, tag="idx")
            nc.sync.dma_start(idx_sb[:], edge_idx_d[:])
            ids_sb = pp.tile([P, NT_TOT], f32, tag="ids")
            nc.sync.dma_start(ids_sb[:], edge_ids_d[:])
            tok_idx_sb = pp.tile([P, c.NVB * EC // 16], i16, tag="tokidx")
            nc.sync.dma_start(tok_idx_sb[:], tok_idx_d[:])
            tok_scat_sb = pp.tile([P, c.NVB * EC // 16], i16, tag="tokscat")
            nc.sync.dma_start(tok_scat_sb[:], tok_scat_d[:])
            iota_f_sb = pp.tile([P, P], f32, tag="iotaf")
            nc.sync.dma_start(iota_f_sb[:], iota_f[:])
            iota_b_sb = pp.tile([P, P], bf16, tag="iotab")
            nc.vector.tensor_copy(iota_b_sb[:], iota_f_sb[:])
            ident = pp.tile([P, P], f32, tag="ident")
            make_identity(nc, ident[:])
            g_ids_sb = pp.tile([P, DBL], f32, tag="gids")
            nc.sync.dma_start(g_ids_sb[:], g_ids_d[:])
            is_end_sb = pp.tile([P, DBL], f32, tag="iend")
            nc.sync.dma_start(is_end_sb[:], is_end_d[:])
            ones_row = pp.tile([1, D], f32, tag="ones_row")
            nc.vector.memset(ones_row[:], 1.0)
            # persistent u table (dinv*x), node-row layout, updated in place
            u_tab = pp.tile([P, DBL, D], f32, tag="u_sb0")

            deg_sb = wp.tile([P, DBL], f32, tag="deg")
            nc.sync.dma_start(deg_sb[:], deg_loc_d[:])
            dinv = pp.tile([P, DBL], f32, tag="dinv")
            rsqrt_refined(dinv, deg_sb, wp, DBL)
            degp_sb = wp.tile([P, c.NVB * ECC], f32, tag="degp")
            nc.sync.dma_start(degp_sb[:], deg_perm_d[:])
            dinvp = pp.tile([P, c.NVB * ECC], f32, tag="dinvp")
            rsqrt_refined(dinvp, degp_sb, wp, c.NVB * ECC)

            W_sb, b_sb = [], []
            for i in range(3):
                w = pp.tile([D, D], f32, tag=f"W{i}")
                nc.sync.dma_start(w[:], Ws[i][:])
                W_sb.append(w)
                b = pp.tile([D, 1], f32, tag=f"b{i}")
                nc.sync.dma_start(b[:], bs[i][:, None])
                b_sb.append(b)
            Wc1_sb = pp.tile([2 * D, D], f32, tag="Wc1")
            nc.sync.dma_start(Wc1_sb[:], Wc1_d[:])
            bc1_sb = pp.tile([D, 1], f32, tag="bc1")
            nc.sync.dma_start(bc1_sb[:], bc1_d[:, None])
            Wc2_sb = pp.tile([D, 2], f32, tag="Wc2")
            nc.sync.dma_start(Wc2_sb[:], Wc2_d[:])
            bc2_sb = pp.tile([2, 1], f32, tag="bc2")
            nc.sync.dma_start(bc2_sb[:], bc2_d[:, None])

            # ---------- embedding: u0 = dinv * emb[tok] ----------------------
            # scatter-add into DRAM u0_loc (TRASH row soaks padding), then a
            # linear DMA brings the table into SBUF.
            zcols = (c.NLP + P) * D // P
            zchunk = 512
            zinit = wp.tile([P, zchunk], f32, tag="zi")
            nc.vector.memset(zinit[:], 0.0)
            zoff = 0
            uflat = u_loc[0][:, :].rearrange("(a b) c -> a (b c)", a=P)
            while zoff < zcols:
                w = min(zchunk, zcols - zoff)
                nc.sync.dma_start(uflat[:, zoff : zoff + w], zinit[:, 0:w])
                zoff += w
            for b in range(c.NVB):
                g = emsgp.tile([P, ECC, D], f32, tag="emsg")
                nc.gpsimd.dma_gather(
                    g[:], emb[b * c.VBS : min((b + 1) * c.VBS, c.V), :],
                    tok_idx_sb[:, b * (EC // 16) : (b + 1) * (EC // 16)],
                    EC, EC, D, elem_step=D, single_packet=False,
                    queue_num=b % nswq,
                )
                sc = emsgp.tile([P, ECC, D], f32, tag="emsg")
                for cc in range(ECC):
                    nc.vector.tensor_scalar(
                        sc[:, cc, 0:D], g[:, cc, :],
                        dinvp[:, b * ECC + cc : b * ECC + cc + 1], None, OP.mult,
                    )
                nc.gpsimd.dma_scatter_add(
                    u_loc[0][:, :], sc[:],
                    tok_scat_sb[:, b * (EC // 16) : (b + 1) * (EC // 16)],
                    EC, EC, D, elem_step=D, single_packet=False,
                    queue_num=b % nswq,
                )
            nc.sync.dma_start(
                u_tab[:, :, :],
                u_loc[0][0 : c.NLP, :].rearrange("(t p) c -> p t c", p=P),
            )
            if not no_coll:
                for q in range(c.NSH):
                    nc.gpsimd.collective_compute(
                        "AllGather", OP.bypass, replica_groups=rg,
                        ins=[u_loc[0][c.qstart[q] : c.qstart[q] + c.qrows[q], :]],
                        outs=[u_full[0][q][:, :]],
                    )

            # ---------- GCN layers -------------------------------------------
            pool_carry = {"g": None, "v": None}
            pool_sum_cat = ppsum.tile([P, c.GB * (D + 1)], f32, tag="plscat")
            pool_max_cat = ppsum.tile([P, c.GB * D], f32, tag="plmcat")
            pool_sum_ps = [
                pool_sum_cat[:, g * (D + 1) : (g + 1) * (D + 1)]
                for g in range(c.GB)
            ]
            pool_max_ps = [
                pool_max_cat[:, g * D : (g + 1) * D] for g in range(c.GB)
            ]
            qend = [(qs + qr) // P for qs, qr in zip(c.qstart, c.qrows)]

            for layer in range(3):
                table = u_full[layer]
                u_sb = u_next = u_tab
                # issue all gather calls (Tile throttles via msg pool bufs)
                msg_tiles = {}
                for cidx, (s, off, nt) in enumerate(CALLS):
                    mf = msgfp.tile([P, c.CALL_TILES, D], f32, tag="msgf")
                    t0g = SHARD_T0[s] + off
                    if lin_gather:
                        nc.sync.dma_start(
                            mf[:, 0:nt, :],
                            table[s][:, :].rearrange(
                                "(t p) c -> p t c", p=P
                            )[:, 0:nt, :],
                        )
                    else:
                        nc.gpsimd.dma_gather(
                            mf[:, 0:nt, :],
                            table[s][:, :],
                            idx_sb[:, t0g * 8 : (t0g + nt) * 8],
                            nt * P, nt * P, D, elem_step=D,
                            single_packet=False,
                            queue_num=cidx % nswq,
                        )
                    m = msgp.tile([P, c.CALL_TILES, D], bf16, tag="msg")
                    if cast_dma:
                        nc.gpsimd.dma_start(m[:, 0:nt, :], mf[:, 0:nt, :])
                    else:
                        nc.scalar.activation(
                            m[:, 0:nt, :], mf[:, 0:nt, :], AF.Identity,
                        )
                    msg_tiles[cidx] = m
                # d-major consumption: one psum per dst block
                for d, ents in CELLS:
                    zp = zpsp.tile([P, D], f32, tag="zp")
                    # self-loop init: zp = I.T @ u_sb[:, d]
                    nc.tensor.matmul(
                        zp[:], ident[:], u_sb[:, d, 0:D],
                        start=True, stop=(not ents),
                    )
                    ntot = sum(e[2] for e in ents)
                    done = 0
                    if no_agg:
                        nc.tensor.matmul(
                            zp[:], ident[:], u_sb[:, d, 0:D],
                            start=False, stop=True,
                        )
                    for (s, ct0, nt) in (() if no_agg else ents):
                        for j in range(nt):
                            cidx, col = T2C[(s, ct0 + j)]
                            tg = SHARD_T0[s] + ct0 + j
                            S = sp.tile([P, P], bf16, tag="S")
                            seng = (
                                nc.gpsimd
                                if (sgen_gp and done % sgen_gp == 0)
                                else nc.vector
                            )
                            seng.tensor_scalar(
                                S[:], iota_b_sb[:],
                                ids_sb[:, tg : tg + 1], None, OP.is_equal,
                            )
                            done += 1
                            nc.tensor.matmul(
                                zp[:], S[:], msg_tiles[cidx][:, col, 0:D],
                                start=False, stop=(done == ntot),
                            )
                    # ---- per-node update for this dst block ----
                    zsc = wp.tile([P, D], f32, tag="zsc")
                    if dve_copy:
                        nc.vector.tensor_scalar(
                            zsc[:], zp[:], dinv[:, d : d + 1], None, OP.mult,
                        )
                    else:
                        nc.scalar.activation(
                            zsc[:], zp[:], AF.Identity,
                            scale=dinv[:, d : d + 1],
                        )
                    tp = psp.tile([D, P], f32, tag="ps")
                    nc.tensor.transpose(tp[:], zsc[:], ident[:])
                    wT = wp.tile([D, P], f32, tag="wT")
                    if dve_copy:
                        nc.vector.tensor_copy(wT[:], tp[:])
                    else:
                        nc.scalar.activation(wT[:], tp[:], AF.Identity)
                    op = psp.tile([D, P], f32, tag="ps")
                    nc.tensor.matmul(op[:], W_sb[layer][:], wT[:], start=True, stop=True)
                    oT = wp.tile([D, P], f32, tag="oT")
                    nc.scalar.activation(oT[:], op[:], AF.Relu, bias=b_sb[layer][:])
                    if layer < 2:
                        bp = psp.tile([P, D], f32, tag="ps")
                        nc.tensor.transpose(bp[:], oT[:], ident[0:D, 0:D])
                        if dve_copy:
                            nc.vector.tensor_scalar(
                                u_next[:, d, 0:D], bp[:], dinv[:, d : d + 1],
                                None, OP.mult,
                            )
                        else:
                            nc.scalar.activation(
                                u_next[:, d, 0:D], bp[:], AF.Identity,
                                scale=dinv[:, d : d + 1],
                            )
                        rows = min(c.NLP, (d + 1) * P) - d * P
                        nc.sync.dma_start(
                            u_loc[layer + 1][d * P : d * P + rows, 0:D],
                            u_next[0:rows, d, 0:D],
                        )
                        if d + 1 in qend and not no_coll:
                            q = qend.index(d + 1)
                            nc.gpsimd.collective_compute(
                                "AllGather", OP.bypass, replica_groups=rg,
                                ins=[u_loc[layer + 1][c.qstart[q] : c.qstart[q] + c.qrows[q], :]],
                                outs=[u_full[layer + 1][q][:, :]],
                            )
                    else:
                        # ---- pooling ----
                        bp = psp.tile([P, D], f32, tag="ps")
                        nc.tensor.transpose(bp[:], oT[:], ident[0:D, 0:D])
                        xf = wp.tile([P, D + 1], f32, tag="xf")
                        nc.scalar.activation(xf[:, 0:D], bp[:], AF.Identity)
                        nc.vector.memset(xf[:, D : D + 1], 1.0)
                        for g in range(c.GB):
                            Sg = sp.tile([P, P], f32, tag="Sg")
                            nc.vector.tensor_scalar(
                                Sg[:], iota_f_sb[:], float(g * P),
                                g_ids_sb[:, d : d + 1], OP.add, OP.is_equal,
                            )
                            nc.tensor.matmul(
                                pool_sum_ps[g], Sg[:], xf[:, 0 : D + 1],
                                start=(d == 0), stop=(d == DBL - 1),
                            )
                        # graph-id row broadcast to [D, P] via PE
                        t1 = psp.tile([1, P], f32, tag="ps")
                        nc.tensor.transpose(
                            t1[:], g_ids_sb[:, d : d + 1], ident[:]
                        )
                        t1s = wp.tile([1, P], f32, tag="t1s")
                        nc.vector.tensor_copy(t1s[:], t1[:])
                        gb_ps = psp.tile([D, P], f32, tag="ps")
                        nc.tensor.matmul(
                            gb_ps[:], ones_row[:], t1s[:], start=True, stop=True
                        )
                        gdb = wp.tile([D, P], f32, tag="gdb")
                        nc.vector.tensor_copy(gdb[:], gb_ps[:])
                        mscan = wp.tile([D, P], f32, tag="mscan")
                        nc.vector.tensor_copy(mscan[:], oT[:])
                        sh = 1
                        while sh < P:
                            msk = wp.tile([D, P], f32, tag="msk")
                            nc.vector.tensor_tensor(
                                msk[:, sh:P], gdb[:, sh:P], gdb[:, 0 : P - sh],
                                OP.is_equal,
                            )
                            tmp = wp.tile([D, P], f32, tag="tmpscan")
                            nc.vector.tensor_tensor(
                                tmp[:, sh:P], mscan[:, 0 : P - sh], msk[:, sh:P],
                                OP.mult,
                            )
                            nc.vector.tensor_tensor(
                                mscan[:, sh:P], mscan[:, sh:P], tmp[:, sh:P], OP.max
                            )
                            sh *= 2
                        if pool_carry["g"] is not None:
                            cmask = wp.tile([D, P], f32, tag="cmask")
                            nc.vector.tensor_scalar(
                                cmask[:], gdb[:], pool_carry["g"][:, 0:1], None,
                                OP.is_equal,
                            )
                            nc.vector.tensor_scalar(
                                cmask[:], cmask[:], pool_carry["v"][:, 0:1], None,
                                OP.mult,
                            )
                            nc.vector.tensor_tensor(
                                mscan[:], mscan[:], cmask[:], OP.max
                            )
                        cg = wp.tile([D, 1], f32, tag="cg", bufs=2)
                        cv = wp.tile([D, 1], f32, tag="cv", bufs=2)
                        nc.vector.tensor_copy(cg[:], gdb[:, P - 1 : P])
                        nc.vector.tensor_copy(cv[:], mscan[:, P - 1 : P])
                        pool_carry = {"g": cg, "v": cv}
                        sc_ps = psp.tile([P, D], f32, tag="ps")
                        nc.tensor.transpose(sc_ps[:], mscan[:], ident[0:D, 0:D])
                        scT = wp.tile([P, D], f32, tag="scT")
                        nc.scalar.activation(scT[:], sc_ps[:], AF.Identity)
                        for g in range(c.GB):
                            Se = sp.tile([P, P], f32, tag="Sg")
                            nc.vector.tensor_scalar(
                                Se[:], iota_f_sb[:], float(g * P),
                                g_ids_sb[:, d : d + 1], OP.add, OP.is_equal,
                            )
                            nc.vector.tensor_scalar(
                                Se[:], Se[:], is_end_sb[:, d : d + 1], None, OP.mult
                            )
                            nc.tensor.matmul(
                                pool_max_ps[g], Se[:], scT[:],
                                start=(d == 0), stop=(d == DBL - 1),
                            )

            # ---------- pool exchange + classifier ---------------------------
            pl = wp.tile([P, c.GB * PCOLS], f32, tag="pl")
            for g in range(c.GB):
                o = g * PCOLS
                nc.vector.tensor_copy(pl[:, o : o + D + 1], pool_sum_ps[g])
                nc.vector.tensor_copy(
                    pl[:, o + D + 1 : o + 2 * D + 1], pool_max_ps[g]
                )
                nc.vector.memset(pl[:, o + 2 * D + 1 : o + PCOLS], 0.0)
            nc.sync.dma_start(pool_loc_d[:, :], pl[:])
            if not no_coll:
                nc.gpsimd.collective_compute(
                    "AllGather", OP.bypass, replica_groups=rg,
                    ins=[pool_loc_d[:, :]], outs=[pool_all_d[:, :]],
                )
            comb = wp.tile([P, c.GB * PCOLS], f32, tag="comb")
            nc.vector.memset(comb[:], 0.0)
            for r in range(c.NC):
                pr = wp.tile([P, c.GB * PCOLS], f32, tag="pr")
                nc.sync.dma_start(pr[:], pool_all_d[r * P : (r + 1) * P, :])
                for g in range(c.GB):
                    o = g * PCOLS
                    nc.vector.tensor_add(
                        comb[:, o : o + D + 1], comb[:, o : o + D + 1],
                        pr[:, o : o + D + 1],
                    )
                    nc.vector.tensor_tensor(
                        comb[:, o + D + 1 : o + 2 * D + 1],
                        comb[:, o + D + 1 : o + 2 * D + 1],
                        pr[:, o + D + 1 : o + 2 * D + 1], OP.max,
                    )
            hT = wp.tile([2 * D, c.GB * P], f32, tag="hT")
            for g in range(c.GB):
                o = g * PCOLS
                cnt = wp.tile([P, 1], f32, tag="cnt")
                nc.vector.tensor_scalar(
                    cnt[:], comb[:, o + D : o + D + 1], 1.0, None, OP.max
                )
                rc = wp.tile([P, 1], f32, tag="rc")
                nc.vector.reciprocal(rc[:], cnt[:])
                t2r = wp.tile([P, 1], f32, tag="t2r")
                nc.vector.tensor_tensor(t2r[:], cnt[:], rc[:], OP.mult)
                nc.vector.tensor_scalar(t2r[:], t2r[:], -1.0, 2.0, OP.mult, OP.add)
                nc.vector.tensor_tensor(cnt[:], rc[:], t2r[:], OP.mult)
                mean = wp.tile([P, D], f32, tag="mean")
                nc.vector.tensor_scalar(
                    mean[:], comb[:, o : o + D], cnt[:, 0:1], None, OP.mult
                )
                mps = psp.tile([D, P], f32, tag="ps")
                nc.tensor.transpose(mps[:], mean[:], ident[:])
                nc.vector.tensor_copy(hT[0:D, g * P : (g + 1) * P], mps[:])
                xps = psp.tile([D, P], f32, tag="ps")
                nc.tensor.transpose(
                    xps[:], comb[:, o + D + 1 : o + 2 * D + 1], ident[:]
                )
                nc.vector.tensor_copy(hT[D : 2 * D, g * P : (g + 1) * P], xps[:])
            h1 = psp.tile([D, c.GB * P], f32, tag="ps")
            nc.tensor.matmul(h1[:], Wc1_sb[:], hT[:], start=True, stop=True)
            h1s = wp.tile([D, c.GB * P], f32, tag="h1s")
            nc.scalar.activation(h1s[:], h1[:], AF.Relu, bias=bc1_sb[:])
            lg = psp.tile([2, c.GB * P], f32, tag="ps")
            nc.tensor.matmul(lg[:], Wc2_sb[:], h1s[:], start=True, stop=True)
            lgs = wp.tile([2, c.GB * P], f32, tag="lgs")
            nc.scalar.activation(lgs[:], lg[:], AF.Identity, bias=bc2_sb[:])
            for g in range(c.GB):
                lt = psp.tile([P, 2], f32, tag="ps")
                nc.tensor.transpose(
                    lt[:], lgs[:, g * P : (g + 1) * P], ident[0:2, 0:2]
                )
                lts = wp.tile([P, 2], f32, tag="lts")
                nc.vector.tensor_copy(lts[:], lt[:])
                rows = min(c.G, (g + 1) * P) - g * P
                nc.sync.dma_start(logits_d[g * P : g * P + rows, :], lts[0:rows, :])

    nc.compile()
    return nc


def make_in_maps(cfg, data, inputs):
    shared = {
        "emb_table": np.asarray(inputs["emb_table"], dtype=np.float32),
        "Wc1": np.asarray(inputs["Wc1"], dtype=np.float32),
        "bc1": np.asarray(inputs["bc1"], dtype=np.float32),
        "Wc2": np.asarray(inputs["Wc2"], dtype=np.float32),
        "bc2": np.asarray(inputs["bc2"], dtype=np.float32),
    }
    for i in range(3):
        shared[f"W{i}"] = np.asarray(inputs[f"W{i}"], dtype=np.float32)
        shared[f"b{i}"] = np.asarray(inputs[f"b{i}"], dtype=np.float32)
    in_maps = []
    for ci in range(cfg.NC):
        m = dict(shared)
        for k, v in data.items():
            m[k] = v[ci]
        in_maps.append(m)
    return in_maps


def kernel(**inputs):
    from concourse.bass_utils import run_bass_kernel_spmd

    x_tokens = np.asarray(inputs["x_tokens"])
    edge_index = np.asarray(inputs["edge_index"])
    batch = np.asarray(inputs["batch"])
    N = x_tokens.shape[0]
    E = edge_index.shape[1]
    V = np.asarray(inputs["emb_table"]).shape[0]
    G = 256
    cfg = Cfg(N, E, V, G)

    meta, data = _preprocess(cfg, x_tokens, edge_index, batch)
    nc = build_program(cfg, meta)
    in_maps = make_in_maps(cfg, data, inputs)
    res = run_bass_kernel_spmd(nc, in_maps, core_ids=list(range(cfg.NC)))
    return np.asarray(res.results[0]["logits"])
